# revision 11
# baseline (speedup 1.0000x reference)
"""Trainium2 Bass kernel for the ContextFusion module.

The reference computes, per batch b:
    diff[i,j,c]  = x[j,c] - x[i,c]
    var[i,c]     = Var_j(diff)              (= Var_j(x[:,c]) -- independent of i)
    f_tilde      = diff * rsqrt(var+eps)
    fc_k         = concat([f_tilde, x[i,c] bcast], -1) * gamma + beta
    w            = softmax_j(fc_k)
    fc_c[i,k]    = sum_j w * fc_k
    out          = relu(fc_c @ W.T)

Algebraic collapse (exact in infinite precision, ~1e-6 rel err in fp32):
  * Var_j(x[j,c] - x[i,c]) = Var_j(x[j,c])            -- shift-invariant
  * softmax_j over the f_tilde half depends only on s[j,c] = x[j,c]*r[c]*g1[c]
    (the -x[i,c]*r*g1 + beta terms are constant in j), so the weights w[j,c]
    are shared by every center point i
  * softmax_j over the f_center half is uniform (value constant in j)
  So with A[c] = sum_j w[j,c]*s[j,c]:
    fc_c[i, c]   = A[c] - x[i,c]*r[c]*g1[c] + b1[c]
    fc_c[i, C+c] = x[i,c]*g2[c] + b2[c]
    out[i, co]   = relu( K[co] + sum_c x[i,c] * Meff[c, co] )
      Meff[c,co] = g2[c]*W[co,C+c] - r[c]*g1[c]*W[co,c]
      K[co]      = sum_c (A[c]+b1[c])*W[co,c] + sum_c b2[c]*W[co,C+c]

Sharding: 8 cores; core k handles batch b = k//4 and output rows
[256*(k%4), 256*(k%4+1)).  Per-channel stats (rg, A) are recomputed per
core for its batch (cheap: one [64,1024] softmax).  The host pre-rotates
and transposes x[b] so each core's row chunk is columns 0:256 of its xt
input (all per-channel reductions are permutation-invariant over j), and
pre-computes every W/gamma/beta-derived constant.

Numerics notes:
  * rsqrt(var+eps)*g1 is computed as exp(-0.5*ln(var+eps) + ln(g1)):
    ln+exp live in one ACT table set (sqrt does not), avoiding a second
    ~2.7us table load.  Requires g1 > 0 (gamma is a LayerNorm-style scale,
    ones in this model).
  * The softmax skips max-subtraction: s = x*rg with x ~ N(0,1) and
    rg ~ 1/std, so |s| <~ 6 and exp(s) is far from fp32 overflow.  The
    reference's max-subtraction is mathematically a no-op.
"""

import os
import sys

sys.path.insert(0, "/opt/trn_rl_repo")

import numpy as np

# Problem constants (hardcoded per spec: x is [2, 1024, 64] fp32).
B, N, C = 2, 1024, 64
EPS = 1e-5
NCORES = 8
CPB = NCORES // B          # cores per batch
R = N // CPB               # output rows per core

# wpack column layout
_W1T_COLS = slice(0, C)            # rows 0:64 = W1^T, row 64 = kconst
_P2_COLS = slice(C, 2 * C)         # rows 0:64 = g2 * W2^T
_LNG1_COL = slice(2 * C, 2 * C + 1)
_EPS_COL = slice(2 * C + 1, 2 * C + 2)
_ZERO_COL = slice(2 * C + 2, 2 * C + 3)
WP_COLS = 2 * C + 3

_NC = None
LAST_RESULTS = None
RUN_KWARGS = {}  # test-harness hook: extra kwargs for run_bass_kernel_spmd


def _build_program():
    import concourse.bacc as bacc
    import concourse.tile as tile
    from concourse import mybir

    f32 = mybir.dt.float32
    AF = mybir.ActivationFunctionType
    ALU = mybir.AluOpType

    # Bacc (not plain Bass): its finalize() runs the wait-splitting passes
    # (move_matmul_waits_to_ldweights / generate_event_semaphores) that the
    # TRN2 1-wait-per-instruction constraint requires.
    nc = bacc.Bacc()
    xt_d = nc.declare_dram_parameter("xt", [C + 1, N], f32, isOutput=False)
    wp_d = nc.declare_dram_parameter("wp", [128, WP_COLS], f32, isOutput=False)
    out_d = nc.declare_dram_parameter("out", [R, C], f32, isOutput=True)

    H = N // 2

    with tile.TileContext(nc) as tc:
        with (
            tc.tile_pool(name="singles", bufs=1) as singles,
            tc.tile_pool(name="small", bufs=2) as small,
            tc.tile_pool(name="kpsp", bufs=1, space="PSUM") as kps_pool,
            tc.tile_pool(name="pso", bufs=2, space="PSUM") as pso_pool,
        ):
            # Dependency-free preamble: ACT exp/ln table warm-up + the "1"
            # element of the augmented A vector.
            dum = singles.tile([1, 1], f32)
            nc.gpsimd.memset(dum, 0.0)
            dum2 = singles.tile([1, 1], f32)
            nc.scalar.activation(out=dum2, in_=dum, func=AF.Exp, bias=dum[0:1, 0:1])
            av = singles.tile([C + 1, 1], f32)
            nc.gpsimd.memset(av[C:C + 1, :], 1.0)
            ones1 = singles.tile([1, 128], f32)
            nc.gpsimd.memset(ones1, 1.0)

            wp = singles.tile([128, WP_COLS], f32)
            nc.sync.dma_start(out=wp, in_=wp_d[:])
            xta = singles.tile([C + 1, N], f32)
            nc.sync.dma_start(out=xta[:, 0:H], in_=xt_d[:, 0:H])
            nc.sync.dma_start(out=xta[:, H:N], in_=xt_d[:, H:N])

            lng1 = wp[0:C, _LNG1_COL]
            eps_c = wp[0:C, _EPS_COL]
            zero_c = wp[0:C, _ZERO_COL]
            zero_p = wp[0:128, _ZERO_COL]

            # Per-channel variance over the N axis (two chunks so the first
            # overlaps the second DMA half).
            stats = small.tile([C, 2, 6], f32, tag="stats")
            nc.vector.bn_stats(out=stats[:, 0, :], in_=xta[0:C, 0:H])
            nc.vector.bn_stats(out=stats[:, 1, :], in_=xta[0:C, H:N])
            mv = small.tile([C, 2], f32, tag="mv")
            nc.vector.bn_aggr(out=mv, in_=stats)

            # rg = g1 * rsqrt(var+eps) = exp(-0.5*ln(var+eps) + ln(g1))
            lnv = small.tile([C, 1], f32, tag="lnv")
            nc.scalar.activation(out=lnv, in_=mv[:, 1:2], func=AF.Ln, bias=eps_c)
            rg = small.tile([C, 1], f32, tag="rg")
            nc.scalar.activation(out=rg, in_=lnv, func=AF.Exp, bias=lng1, scale=-0.5)

            # softmax sums over j of s = x*rg (no max-shift, see header):
            # Z = sum exp(s), SX = sum x*exp(s), halves pipelined ACT->DVE.
            e = singles.tile([C, N], f32)
            zx = small.tile([C, 2], f32, tag="zx")
            nc.scalar.activation(
                out=e[:, 0:H], in_=xta[0:C, 0:H], func=AF.Exp,
                bias=zero_c, scale=rg, accum_out=zx[:, 0:1],
            )
            nc.scalar.activation(
                out=e[:, H:N], in_=xta[0:C, H:N], func=AF.Exp,
                bias=zero_c, scale=rg, accum_out=zx[:, 1:2],
            )
            # (tensor_tensor_reduce crashes the device -- NRT status 101 --
            # so SX = sum(e*x) is a mul per half + one reduce instead.)
            ex = singles.tile([C, N], f32)
            nc.vector.tensor_mul(out=ex[:, 0:H], in0=e[:, 0:H], in1=xta[0:C, 0:H])
            nc.vector.tensor_mul(out=ex[:, H:N], in0=e[:, H:N], in1=xta[0:C, H:N])
            sxt = small.tile([C, 1], f32, tag="sxt")
            nc.vector.reduce_sum(out=sxt, in_=ex, axis=mybir.AxisListType.X)
            zsum = small.tile([C, 1], f32, tag="zsum")
            nc.vector.tensor_scalar_add(out=zsum, in0=zx[:, 0:1], scalar1=zx[:, 1:2])
            zr = small.tile([C, 1], f32, tag="zr")
            nc.vector.reciprocal(out=zr, in_=zsum)
            # A = rg * SX / Z
            nc.vector.tensor_scalar(
                out=av[0:C, :], in0=sxt, scalar1=zr, scalar2=rg,
                op0=ALU.mult, op1=ALU.mult,
            )

            # Meff[c,co] = p2[c,co] - rg[c]*W1T[c,co]
            t1 = small.tile([C, C], f32, tag="t1")
            nc.vector.tensor_scalar_mul(out=t1, in0=wp[0:C, _W1T_COLS], scalar1=rg)
            meff = singles.tile([C, C], f32)
            nc.vector.tensor_sub(out=meff, in0=wp[0:C, _P2_COLS], in1=t1)

            # K row = [A; 1]^T @ [W1T; kconst] in PSUM, copied to SBUF.
            # (A matmul writing PSUM at partition base 64 crashes the device
            # -- NRT_EXEC_UNIT_UNRECOVERABLE -- so keep everything base 0
            # and add the K row with a second accumulating matmul below.)
            kpt = kps_pool.tile([1, C], f32, tag="kpt")
            nc.tensor.matmul(
                kpt, lhsT=av, rhs=wp[0:C + 1, _W1T_COLS], start=True, stop=True,
            )
            krow = singles.tile([1, C], f32)
            nc.scalar.copy(out=krow, in_=kpt)

            # out rows = relu(x_chunk^T @ Meff + 1^T @ K)
            for h in range(R // 128):
                po = pso_pool.tile([128, C], f32, tag="po")
                nc.tensor.matmul(
                    po, lhsT=xta[0:C, h * 128:(h + 1) * 128], rhs=meff,
                    start=True, stop=False,
                )
                nc.tensor.matmul(
                    po, lhsT=ones1, rhs=krow, start=False, stop=True,
                )
                osb = small.tile([128, C], f32, tag="osb")
                nc.scalar.activation(out=osb, in_=po, func=AF.Relu, bias=zero_p)
                nc.sync.dma_start(out=out_d[h * 128:(h + 1) * 128, :], in_=osb)

    return nc


def _get_nc():
    global _NC
    if _NC is None:
        _NC = _build_program()
        # run_bass_via_pjrt does not finalize a prebuilt module; Bacc's
        # finalize() must run so its wait-splitting/reg-alloc passes apply
        # before serialization to neuronx-cc.
        _NC.finalize()
    return _NC


def _host_prep(x, gamma, beta, W):
    g = np.asarray(gamma, dtype=np.float32).reshape(-1)
    bt = np.asarray(beta, dtype=np.float32).reshape(-1)
    W = np.asarray(W, dtype=np.float32)

    g1c, g2c = g[:C], g[C:]
    b1c, b2c = bt[:C], bt[C:]
    W1, W2 = W[:, :C], W[:, C:]

    wp = np.zeros((128, WP_COLS), dtype=np.float32)
    wp[0:C, _W1T_COLS] = W1.T
    wp[C, _W1T_COLS] = W1 @ b1c + W2 @ b2c          # kconst row
    wp[0:C, _P2_COLS] = g2c[:, None] * W2.T
    wp[0:C, _LNG1_COL] = np.log(g1c)[:, None]
    wp[0:C, _EPS_COL] = EPS
    return np.ascontiguousarray(wp)


def kernel(x, gamma, beta, W):
    global LAST_RESULTS
    from concourse.bass_utils import run_bass_kernel_spmd

    x = np.ascontiguousarray(np.asarray(x, dtype=np.float32))
    wp = _host_prep(x, gamma, beta, W)

    nc = _get_nc()
    in_maps = []
    for k in range(NCORES):
        b, q = divmod(k, CPB)
        i0 = q * R
        xt = np.empty((C + 1, N), dtype=np.float32)
        # columns j hold x[b, (i0+j) % N, :]; per-channel reductions over j
        # are permutation-invariant, and columns 0:R are this core's rows.
        xt[0:C, :] = np.roll(x[b], -i0, axis=0).T
        xt[C, :] = 1.0
        in_maps.append({"xt": xt, "wp": wp})

    res = run_bass_kernel_spmd(
        nc, in_maps, core_ids=list(range(NCORES)),
        trace=bool(os.environ.get("KERNEL_TRACE")),
        **RUN_KWARGS,
    )
    LAST_RESULTS = res

    out = np.empty((B, N, C), dtype=np.float32)
    for k in range(NCORES):
        b, q = divmod(k, CPB)
        i0 = q * R
        out[b, i0:i0 + R] = res.results[k]["out"]
    return out


# revision 16
# speedup vs baseline: 1.1231x; 1.1231x over previous
"""Trainium2 Bass kernel for the ContextFusion module.

The reference computes, per batch b:
    diff[i,j,c]  = x[j,c] - x[i,c]
    var[i,c]     = Var_j(diff)              (= Var_j(x[:,c]) -- independent of i)
    f_tilde      = diff * rsqrt(var+eps)
    fc_k         = concat([f_tilde, x[i,c] bcast], -1) * gamma + beta
    w            = softmax_j(fc_k)
    fc_c[i,k]    = sum_j w * fc_k
    out          = relu(fc_c @ W.T)

Algebraic collapse (exact in infinite precision, ~1e-6 rel err in fp32):
  * Var_j(x[j,c] - x[i,c]) = Var_j(x[j,c])            -- shift-invariant
  * softmax_j over the f_tilde half depends only on s[j,c] = x[j,c]*r[c]*g1[c]
    (the -x[i,c]*r*g1 + beta terms are constant in j), so the weights w[j,c]
    are shared by every center point i
  * softmax_j over the f_center half is uniform (value constant in j)
  So with A[c] = sum_j w[j,c]*s[j,c]:
    fc_c[i, c]   = A[c] - x[i,c]*r[c]*g1[c] + b1[c]
    fc_c[i, C+c] = x[i,c]*g2[c] + b2[c]
    out[i, co]   = relu( K[co] + sum_c x[i,c] * Meff[c, co] )
      Meff[c,co] = g2[c]*W[co,C+c] - r[c]*g1[c]*W[co,c]
      K[co]      = sum_c (A[c]+b1[c])*W[co,c] + sum_c b2[c]*W[co,C+c]

Sharding: 8 cores; core k handles batch b = k//4 and output rows
[256*(k%4), 256*(k%4+1)).  Per-channel stats (rg, A) are recomputed per
core for its batch (cheap: one [64,1024] softmax).  The host pre-rotates
and transposes x[b] so each core's row chunk is columns 0:256 of its xt
input (all per-channel reductions are permutation-invariant over j), and
pre-computes every W/gamma/beta-derived constant.

Numerics notes:
  * rsqrt(var+eps)*g1 is computed as exp(-0.5*ln(var+eps) + ln(g1)):
    ln+exp live in one ACT table set (sqrt does not), avoiding a second
    ~2.7us table load.  Requires g1 > 0 (gamma is a LayerNorm-style scale,
    ones in this model).
  * The softmax skips max-subtraction: s = x*rg with x ~ N(0,1) and
    rg ~ 1/std, so |s| <~ 6 and exp(s) is far from fp32 overflow.  The
    reference's max-subtraction is mathematically a no-op.
"""

import os
import sys

sys.path.insert(0, "/opt/trn_rl_repo")

import numpy as np

# Problem constants (hardcoded per spec: x is [2, 1024, 64] fp32).
B, N, C = 2, 1024, 64
EPS = 1e-5
NCORES = 8
CPB = NCORES // B          # cores per batch
R = N // CPB               # output rows per core

# wpack column layout
_W1T_COLS = slice(0, C)            # rows 0:64 = W1^T, row 64 = kconst
_P2_COLS = slice(C, 2 * C)         # rows 0:64 = g2 * W2^T
_LNG1_COL = slice(2 * C, 2 * C + 1)
_EPS_COL = slice(2 * C + 1, 2 * C + 2)
_ZERO_COL = slice(2 * C + 2, 2 * C + 3)
WP_COLS = 2 * C + 3

_NC = None
LAST_RESULTS = None
RUN_KWARGS = {}  # test-harness hook: extra kwargs for run_bass_kernel_spmd


def _build_program():
    import concourse.bacc as bacc
    import concourse.tile as tile
    from concourse import mybir
    from concourse.vector_clock import ScopedClock

    f32 = mybir.dt.float32
    AF = mybir.ActivationFunctionType
    ALU = mybir.AluOpType

    class LeanTileContext(tile.TileContext):
        """TileContext whose kernel tail skips the semaphore-clear pass and
        second all-engine barrier (~7us on HW).  The Bass preamble clears
        all semaphores at kernel entry, so a stale end-state is safe across
        executions."""

        def _drain_and_barrier(self, tick_clock, wait_clock):
            drain_inst = self.nc.sync.drain()
            wait_clock.add_sem_waits(
                drain_inst.ins, ScopedClock({None: tick_clock.global_clock})
            )
            self.nc.all_engine_barrier()
            popped = self.nc._tile_sem_poison_stack.pop()
            assert popped is self._sem_poison

    # Bacc (not plain Bass): its finalize() runs the wait-splitting passes
    # (move_matmul_waits_to_ldweights / generate_event_semaphores) that the
    # TRN2 1-wait-per-instruction constraint requires.  partition_id is
    # unused -- disabling it removes a ~1.2us 5-engine register-load
    # preamble.
    nc = bacc.Bacc(enable_partition_id=False)
    xt_d = nc.declare_dram_parameter("xt", [C + 1, N], f32, isOutput=False)
    wp_d = nc.declare_dram_parameter("wp", [128, WP_COLS], f32, isOutput=False)
    out_d = nc.declare_dram_parameter("out", [R, C], f32, isOutput=True)

    H = N // 2

    with LeanTileContext(nc) as tc:
        with (
            tc.tile_pool(name="singles", bufs=1) as singles,
            tc.tile_pool(name="small", bufs=2) as small,
            tc.tile_pool(name="kpsp", bufs=1, space="PSUM") as kps_pool,
            tc.tile_pool(name="pso", bufs=2, space="PSUM") as pso_pool,
        ):
            # Dependency-free preamble: ACT exp/ln table warm-up + the "1"
            # element of the augmented A vector.
            dum = singles.tile([1, 1], f32)
            nc.gpsimd.memset(dum, 0.0)
            dum2 = singles.tile([1, 1], f32)
            nc.scalar.activation(out=dum2, in_=dum, func=AF.Exp, bias=dum[0:1, 0:1])
            av = singles.tile([C + 1, 1], f32)
            nc.gpsimd.memset(av[C:C + 1, :], 1.0)
            ones1 = singles.tile([1, 128], f32)
            nc.gpsimd.memset(ones1, 1.0)

            # Input DMAs split across both HWDGE rings (sync=qSP, scalar=qAct)
            # so the two xt halves transfer in parallel.
            xta = singles.tile([C + 1, N], f32)
            nc.sync.dma_start(out=xta[:, 0:H], in_=xt_d[:, 0:H])
            nc.scalar.dma_start(out=xta[:, H:N], in_=xt_d[:, H:N])
            wp = singles.tile([128, WP_COLS], f32)
            nc.sync.dma_start(out=wp, in_=wp_d[:])

            lng1 = wp[0:C, _LNG1_COL]
            eps_c = wp[0:C, _EPS_COL]
            zero_c = wp[0:C, _ZERO_COL]
            zero_p = wp[0:128, _ZERO_COL]

            # Per-channel variance over the N axis (two chunks so the first
            # overlaps the second DMA half).
            stats = small.tile([C, 2, 6], f32, tag="stats")
            nc.vector.bn_stats(out=stats[:, 0, :], in_=xta[0:C, 0:H])
            nc.vector.bn_stats(out=stats[:, 1, :], in_=xta[0:C, H:N])
            mv = small.tile([C, 2], f32, tag="mv")
            nc.vector.bn_aggr(out=mv, in_=stats)

            # rg = g1 * rsqrt(var+eps) = exp(-0.5*ln(var+eps) + ln(g1))
            lnv = small.tile([C, 1], f32, tag="lnv")
            nc.scalar.activation(out=lnv, in_=mv[:, 1:2], func=AF.Ln, bias=eps_c)
            rg = small.tile([C, 1], f32, tag="rg")
            nc.scalar.activation(out=rg, in_=lnv, func=AF.Exp, bias=lng1, scale=-0.5)

            # softmax sums over j of s = x*rg (no max-shift, see header):
            # Z = sum exp(s), SX = sum x*exp(s), halves pipelined ACT->DVE.
            e = singles.tile([C, N], f32)
            zx = small.tile([C, 2], f32, tag="zx")
            nc.scalar.activation(
                out=e[:, 0:H], in_=xta[0:C, 0:H], func=AF.Exp,
                bias=zero_c, scale=rg, accum_out=zx[:, 0:1],
            )
            nc.scalar.activation(
                out=e[:, H:N], in_=xta[0:C, H:N], func=AF.Exp,
                bias=zero_c, scale=rg, accum_out=zx[:, 1:2],
            )
            # (tensor_tensor_reduce crashes the device -- NRT status 101 --
            # so SX = sum(e*x) is a mul per half + one reduce instead.)
            ex = singles.tile([C, N], f32)
            nc.vector.tensor_mul(out=ex[:, 0:H], in0=e[:, 0:H], in1=xta[0:C, 0:H])
            nc.vector.tensor_mul(out=ex[:, H:N], in0=e[:, H:N], in1=xta[0:C, H:N])
            sxt = small.tile([C, 1], f32, tag="sxt")
            nc.vector.reduce_sum(out=sxt, in_=ex, axis=mybir.AxisListType.X)
            zsum = small.tile([C, 1], f32, tag="zsum")
            nc.vector.tensor_scalar_add(out=zsum, in0=zx[:, 0:1], scalar1=zx[:, 1:2])
            zr = small.tile([C, 1], f32, tag="zr")
            nc.vector.reciprocal(out=zr, in_=zsum)
            # A = rg * SX / Z
            nc.vector.tensor_scalar(
                out=av[0:C, :], in0=sxt, scalar1=zr, scalar2=rg,
                op0=ALU.mult, op1=ALU.mult,
            )

            # Meff[c,co] = p2[c,co] - rg[c]*W1T[c,co]
            t1 = small.tile([C, C], f32, tag="t1")
            nc.vector.tensor_scalar_mul(out=t1, in0=wp[0:C, _W1T_COLS], scalar1=rg)
            meff = singles.tile([C, C], f32)
            nc.vector.tensor_sub(out=meff, in0=wp[0:C, _P2_COLS], in1=t1)

            # K row = [A; 1]^T @ [W1T; kconst] in PSUM, copied to SBUF.
            # (A matmul writing PSUM at partition base 64 crashes the device
            # -- NRT_EXEC_UNIT_UNRECOVERABLE -- so keep everything base 0
            # and add the K row with a second accumulating matmul below.)
            kpt = kps_pool.tile([1, C], f32, tag="kpt")
            nc.tensor.matmul(
                kpt, lhsT=av, rhs=wp[0:C + 1, _W1T_COLS], start=True, stop=True,
            )
            krow = singles.tile([1, C], f32)
            nc.scalar.copy(out=krow, in_=kpt)

            # out rows = relu(x_chunk^T @ Meff + 1^T @ K)
            for h in range(R // 128):
                po = pso_pool.tile([128, C], f32, tag="po")
                nc.tensor.matmul(
                    po, lhsT=xta[0:C, h * 128:(h + 1) * 128], rhs=meff,
                    start=True, stop=False,
                )
                nc.tensor.matmul(
                    po, lhsT=ones1, rhs=krow, start=False, stop=True,
                )
                osb = small.tile([128, C], f32, tag="osb")
                nc.scalar.activation(out=osb, in_=po, func=AF.Relu, bias=zero_p)
                dma_eng = nc.sync if h == 0 else nc.scalar
                dma_eng.dma_start(out=out_d[h * 128:(h + 1) * 128, :], in_=osb)

    return nc


def _get_nc():
    global _NC
    if _NC is None:
        _NC = _build_program()
        # run_bass_via_pjrt does not finalize a prebuilt module; Bacc's
        # finalize() must run so its wait-splitting/reg-alloc passes apply
        # before serialization to neuronx-cc.
        #
        # During finalize, pin ACT table selection to the one set that
        # contains ln+exp+relu+copy ("natural_log_exp_and_others").  The
        # default chooser picks the first set per function (exp->set 0,
        # ln->set 5), causing three ~1.3us table loads instead of one.
        # Positions are preserved (entries emptied, not removed) because
        # act_func_set_id is the index into act_info.json.
        import concourse.bacc as bacc_mod

        orig = bacc_mod.get_activation_tables

        def pinned(arch):
            return {
                name: (funcs if name == "natural_log_exp_and_others" else set())
                for name, funcs in orig(arch).items()
            }

        bacc_mod.get_activation_tables = pinned
        try:
            _NC.finalize()
        finally:
            bacc_mod.get_activation_tables = orig
    return _NC


def _host_prep(x, gamma, beta, W):
    g = np.asarray(gamma, dtype=np.float32).reshape(-1)
    bt = np.asarray(beta, dtype=np.float32).reshape(-1)
    W = np.asarray(W, dtype=np.float32)

    g1c, g2c = g[:C], g[C:]
    b1c, b2c = bt[:C], bt[C:]
    W1, W2 = W[:, :C], W[:, C:]

    wp = np.zeros((128, WP_COLS), dtype=np.float32)
    wp[0:C, _W1T_COLS] = W1.T
    wp[C, _W1T_COLS] = W1 @ b1c + W2 @ b2c          # kconst row
    wp[0:C, _P2_COLS] = g2c[:, None] * W2.T
    wp[0:C, _LNG1_COL] = np.log(g1c)[:, None]
    wp[0:C, _EPS_COL] = EPS
    return np.ascontiguousarray(wp)


def kernel(x, gamma, beta, W):
    global LAST_RESULTS
    from concourse.bass_utils import run_bass_kernel_spmd

    x = np.ascontiguousarray(np.asarray(x, dtype=np.float32))
    wp = _host_prep(x, gamma, beta, W)

    nc = _get_nc()
    in_maps = []
    for k in range(NCORES):
        b, q = divmod(k, CPB)
        i0 = q * R
        xt = np.empty((C + 1, N), dtype=np.float32)
        # columns j hold x[b, (i0+j) % N, :]; per-channel reductions over j
        # are permutation-invariant, and columns 0:R are this core's rows.
        xt[0:C, :] = np.roll(x[b], -i0, axis=0).T
        xt[C, :] = 1.0
        in_maps.append({"xt": xt, "wp": wp})

    res = run_bass_kernel_spmd(
        nc, in_maps, core_ids=list(range(NCORES)),
        trace=bool(os.environ.get("KERNEL_TRACE")),
        **RUN_KWARGS,
    )
    LAST_RESULTS = res

    out = np.empty((B, N, C), dtype=np.float32)
    for k in range(NCORES):
        b, q = divmod(k, CPB)
        i0 = q * R
        out[b, i0:i0 + R] = res.results[k]["out"]
    return out
